# revision 5
# baseline (speedup 1.0000x reference)
"""NeuralSDE forecasting kernel for 8x Trainium2 NeuronCores (Bass/Tile).

Data-parallel over batch B=256 across 8 cores (32 batch elems per core).
The per-core scan runs feature-major: state y.T lives in [128, 4*32] SBUF
tiles; column block k holds features 128k..128k+128 of the 32 local batch
columns. out = lhsT.T @ rhs with weight tiles stationary.

Precision: the 255-step recurrence amplifies weight-rounding error ~200x,
so weights are split W = W_hi + W_lo (both bf16) and the state y into
y_hi + y_lo; the product uses three bf16 matmuls (y_hi@W_hi + y_lo@W_hi +
y_hi@W_lo, fp32 PSUM accumulate), restoring ~fp32 accuracy.

Matmul structure (v2): all three product terms accumulate into the SAME
contiguous 32-col psum block per m-chunk. The W_hi matmul streams
[y_hi|y_lo] (N=64) and its out AP has a 0-step middle dim, so the second
32-col half lands on the same psum columns and accumulates via the
per-element has_written bit (verified on HW: rel err 2e-7, and the
contiguous dst is ~17ns/MM faster than a scattered one). The u_t control
projection (x~_t @ [W1x;b1]) and the b2/bg biases are ALSO matmuls into
the same psum group (rank-33 / rank-2 stationaries), so no DVE fold is
needed anywhere: ACT reads tanh(psum) directly. Per-step DVE drops from
12 ops to 6, and the PE stays busy through the end-of-step DVE chain by
front-running the next step's state-independent matmuls (u, biases).

sigmoid(x) = 0.5*(1+tanh(x/2)) keeps the scan on the Tanh ACT table; the
0.5 factors are folded into the host-prescaled dW.
"""

import os
import sys

sys.path.insert(0, "/opt/trn_rl_repo")

import numpy as np
import ml_dtypes

import concourse.bass as bass
import concourse.bacc as bacc
import concourse.mybir as mybir
import concourse.tile as tile
from concourse.bass_utils import run_bass_kernel_spmd

B, T, C, H, O = 256, 256, 32, 512, 32
OUT_TIME = 32
NCORES = 8
BL = B // NCORES  # 32 batch elements per core
NT = int(os.environ.get("BASS_NT", T - 1))  # 255 scan steps
SAVE0 = NT - OUT_TIME  # first step whose y_next lands in the output tail
KC = H // 128  # 4 feature chunks
F32 = mybir.dt.float32
BF16 = mybir.dt.bfloat16
BF = ml_dtypes.bfloat16

Tanh = mybir.ActivationFunctionType.Tanh
Relu = mybir.ActivationFunctionType.Relu
Identity = mybir.ActivationFunctionType.Identity

_BUILT = None


def _h2(ap):  # [128, 32] psum block -> [128, 2, 32] with 0-step middle dim
    return ap.rearrange("p (h q) -> p h q", h=1).broadcast_to((128, 2, 32))


def _build_nc():
    nc = bacc.Bacc("TRN2", target_bir_lowering=False, debug=False)

    # --- DRAM I/O (per-core shards; weights replicated) ---
    # control path for all t: cols t*64..t*64+64 = [x~hi_t | x~lo_t], bf16
    NTP = T  # 256 t-slots (255 used)
    d_xall = nc.dram_tensor("xall2", [C + 1, NTP * 2 * BL], BF16, kind="ExternalInput")
    d_x0 = nc.dram_tensor("x0", [C + 1, BL], F32, kind="ExternalInput")
    d_dw = nc.dram_tensor("dw", [NT, 128, KC * BL], F32, kind="ExternalInput")
    wnames = ["w1y", "w2", "wg"]
    d_w = {
        (n, p): nc.dram_tensor(f"{n}_{p}", [128, KC * H], BF16, kind="ExternalInput")
        for n in wnames
        for p in ("hi", "lo")
    }
    d_w1b = {
        p: nc.dram_tensor(f"w1b_{p}", [C + 1, H], BF16, kind="ExternalInput")
        for p in ("hi", "lo")
    }
    d_wini = nc.dram_tensor("wini", [C + 1, H], F32, kind="ExternalInput")
    d_b2 = nc.dram_tensor("bias2", [2, H], BF16, kind="ExternalInput")
    d_bg = nc.dram_tensor("biasg", [2, H], BF16, kind="ExternalInput")
    d_ones = nc.dram_tensor("ones2", [2, BL], BF16, kind="ExternalInput")
    d_wh1 = nc.dram_tensor("wh1", [128, KC * H], F32, kind="ExternalInput")
    d_wh2 = nc.dram_tensor("wh2", [128, KC * O], F32, kind="ExternalInput")
    d_bh1 = nc.dram_tensor("bh1t", [128, KC], F32, kind="ExternalInput")
    d_bh2 = nc.dram_tensor("bh2t", [O, 1], F32, kind="ExternalInput")
    d_out = nc.dram_tensor("out", [O, OUT_TIME * BL], F32, kind="ExternalOutput")

    with tile.TileContext(nc) as tc:
        with (
            tc.tile_pool(name="const", bufs=1) as const,
            tc.tile_pool(name="dwp", bufs=8) as dwp,
            tc.tile_pool(name="yp", bufs=4) as yp,
            tc.tile_pool(name="tmp", bufs=8) as tmp,
            tc.tile_pool(name="pp", bufs=2, space="PSUM") as pp,
            tc.tile_pool(name="ph", bufs=1, space="PSUM") as ph,
        ):
            # --- resident weights ---
            w_s = {}
            for key, d in d_w.items():
                w_s[key] = const.tile(
                    [128, KC * H], BF16, tag=f"{key[0]}_{key[1]}",
                    name=f"{key[0]}_{key[1]}_s",
                )
                nc.sync.dma_start(out=w_s[key][:], in_=d[:])
            w1b_s = {}
            for p, d in d_w1b.items():
                w1b_s[p] = const.tile([C + 1, H], BF16, tag=f"w1b{p}", name=f"w1b_{p}_s")
                nc.sync.dma_start(out=w1b_s[p][:], in_=d[:])
            wini = const.tile([C + 1, H], F32, tag="wini")
            bias2 = const.tile([2, H], BF16, tag="bias2")
            biasg = const.tile([2, H], BF16, tag="biasg")
            ones2 = const.tile([2, BL], BF16, tag="ones2")
            wh1 = const.tile([128, KC * H], F32, tag="wh1")
            wh2 = const.tile([128, KC * O], F32, tag="wh2")
            bh1 = const.tile([128, KC], F32, tag="bh1")
            bh2 = const.tile([O, 1], F32, tag="bh2")
            x0 = const.tile([C + 1, BL], F32, tag="x0")
            xall = const.tile([C + 1, NTP * 2 * BL], BF16, tag="xall2")
            slab = const.tile([128, OUT_TIME * 128], F32, tag="slab")
            rT = const.tile([128, KC * 1024], F32, tag="rT")
            outs = const.tile([O, OUT_TIME * BL], F32, tag="outs")
            for dst, src in [
                (wini, d_wini), (bias2, d_b2), (biasg, d_bg), (ones2, d_ones),
                (wh1, d_wh1), (wh2, d_wh2), (bh1, d_bh1), (bh2, d_bh2),
                (x0, d_x0), (xall, d_xall),
            ]:
                nc.sync.dma_start(out=dst[:], in_=src[:])

            def wsl(n, p, k, m):  # lhsT tile (k, m) of weight n, part p
                return w_s[(n, p)][:, k * H + m * 128 : k * H + (m + 1) * 128]

            # --- z0 (fp32, one-off) ---
            ps0 = ph.tile([128, 512], F32, tag="ph1", name="ps0")
            for m in range(KC):
                nc.tensor.matmul(
                    ps0[:, m * BL : (m + 1) * BL],
                    wini[:, m * 128 : (m + 1) * 128], x0[:],
                    start=(m == 0), stop=(m == KC - 1),
                )
            y_t = yp.tile([128, KC * BL], F32, tag="y")
            nc.vector.tensor_copy(y_t[:], ps0[:, 0:128])
            y = y_t[:]
            yhl_t = tmp.tile([128, KC * 2 * BL], BF16, tag="yhl", name="yhl_init")
            nc.vector.tensor_copy(yhl_t[:, 0:128], y)
            nc.vector.tensor_sub(yhl_t[:, 128:256], y, yhl_t[:, 0:128])
            yhl = yhl_t

            # --- scan ---
            for t in range(NT):
                psA = pp.tile([128, 256], F32, tag="psA", name=f"psA_{t}")
                psC = pp.tile([128, 256], F32, tag="psC", name=f"psC_{t}")
                psB = pp.tile([128, 256], F32, tag="psB", name=f"psB_{t}")
                dw_t = dwp.tile([128, KC * BL], F32, tag="dw", name=f"dw_{t}")
                nc.sync.dma_start(out=dw_t[:], in_=d_dw[t])

                xsl2 = xall[:, t * 64 : t * 64 + 64].rearrange(
                    "p (h q) -> p h q", h=2
                )
                xhi = xall[:, t * 64 : t * 64 + 32]
                yv = yhl[:].rearrange("p (h q) -> p h q", h=2)

                # dep-free front-runners: u_t into psA, biases into psC/psB.
                # These issue while the previous step's tail DVE chain runs.
                for m in range(KC):
                    nc.tensor.matmul(
                        _h2(psA[:, m * BL : (m + 1) * BL]),
                        w1b_s["hi"][:, m * 128 : (m + 1) * 128], xsl2,
                        start=(m == 0), stop=False,
                    )
                    nc.tensor.matmul(
                        psA[:, m * BL : (m + 1) * BL],
                        w1b_s["lo"][:, m * 128 : (m + 1) * 128], xhi,
                        start=False, stop=False,
                    )
                for m in range(KC):
                    nc.tensor.matmul(
                        psC[:, m * BL : (m + 1) * BL],
                        biasg[:, m * 128 : (m + 1) * 128], ones2[:],
                        start=(m == 0), stop=False,
                    )
                    nc.tensor.matmul(
                        psB[:, m * BL : (m + 1) * BL],
                        bias2[:, m * 128 : (m + 1) * 128], ones2[:],
                        start=(m == 0), stop=False,
                    )

                def prod_group(ps, wname, rhs_hl, rhs_v):
                    # lo pass first (needs only the hi half of the state),
                    # then the N=64 hi pass with 0-step dst (needs both).
                    for m in range(KC):
                        for k in range(KC):
                            nc.tensor.matmul(
                                ps[:, m * BL : (m + 1) * BL],
                                wsl(wname, "lo", k, m),
                                rhs_hl[:, k * BL : (k + 1) * BL],
                                start=False, stop=False,
                            )
                    for m in range(KC):
                        for k in range(KC):
                            nc.tensor.matmul(
                                _h2(ps[:, m * BL : (m + 1) * BL]),
                                wsl(wname, "hi", k, m),
                                rhs_v[:, :, k * BL : (k + 1) * BL],
                                start=False,
                                stop=(m == KC - 1 and k == KC - 1),
                            )

                # h = tanh(y@W1y + u)
                prod_group(psA, "w1y", yhl[:], yv)
                hhl = tmp.tile([128, KC * 2 * BL], BF16, tag="hhl", name=f"hhl_{t}")
                nc.scalar.activation(hhl[:, 0:128], psA[:, 0:128], Tanh)
                h = tmp.tile([128, KC * BL], F32, tag="h", name=f"h_{t}")
                nc.scalar.activation(h[:], psA[:, 0:128], Tanh)
                nc.vector.tensor_sub(hhl[:, 128:256], h[:], hhl[:, 0:128])

                # tau = tanh((y@Wg + bg)/2)  (sigmoid fold)
                prod_group(psC, "wg", yhl[:], yv)
                tau = tmp.tile([128, KC * BL], F32, tag="tau", name=f"tau_{t}")
                nc.scalar.activation(tau[:], psC[:, 0:128], Tanh, scale=0.5)
                # t1 = (tau + 1) * dw ;  dw pre-scaled by 0.5*sqrt(dt)/dt
                t1 = tmp.tile([128, KC * BL], F32, tag="t1", name=f"t1_{t}")
                nc.vector.scalar_tensor_tensor(
                    t1[:], tau[:], 1.0, dw_t[:],
                    mybir.AluOpType.add, mybir.AluOpType.mult,
                )
                yh2 = tmp.tile([128, KC * BL], F32, tag="yh2", name=f"yh2_{t}")
                nc.vector.tensor_add(yh2[:], y, t1[:])

                # f = tanh(h@W2 + b2)
                hv = hhl[:].rearrange("p (h q) -> p h q", h=2)
                prod_group(psB, "w2", hhl[:], hv)
                f = tmp.tile([128, KC * BL], F32, tag="f", name=f"f_{t}")
                nc.scalar.activation(f[:], psB[:, 0:128], Tanh)

                # y_next = (y + t1) + f ; tail states land in the slab.
                if t >= SAVE0:
                    y2 = slab[:, (t - SAVE0) * 128 : (t - SAVE0 + 1) * 128]
                else:
                    y2_t = yp.tile([128, KC * BL], F32, tag="y", name=f"y_{t}")
                    y2 = y2_t[:]
                yhl_t = tmp.tile([128, KC * 2 * BL], BF16, tag="yhl", name=f"yhl_{t}")
                nc.vector.tensor_add(yhl_t[:, 0:128], yh2[:], f[:])
                nc.vector.tensor_add(y2, yh2[:], f[:])
                nc.vector.tensor_sub(yhl_t[:, 128:256], y2, yhl_t[:, 0:128])
                yhl = yhl_t
                y = y2

            # --- head (fp32): out = relu(z_tail@Wh1 + bh1) @ Wh2 + bh2 ---
            # slab columns: s*128 + k*32 + b  (s = tail step, k = feat chunk)
            slab_r = slab[:].rearrange(
                "p (s k b) -> p s k b", s=OUT_TIME, k=KC, b=BL
            )
            for m in range(KC):
                for hf in range(2):
                    ps1 = ph.tile([128, 512], F32, tag="ph1", name=f"ps1_{m}_{hf}")
                    for k in range(KC):
                        nc.tensor.matmul(
                            ps1[:],
                            wh1[:, k * H + m * 128 : k * H + (m + 1) * 128],
                            slab_r[:, hf * 16 : (hf + 1) * 16, k, :],
                            start=(k == 0), stop=(k == KC - 1),
                        )
                    nc.scalar.activation(
                        rT[:, m * 1024 + hf * 512 : m * 1024 + (hf + 1) * 512],
                        ps1[:], Relu, bias=bh1[:, m : m + 1],
                    )
            for hf in range(2):
                ps2 = ph.tile([O, 512], F32, tag="ph2", name=f"ps2_{hf}")
                for m in range(KC):
                    nc.tensor.matmul(
                        ps2[:],
                        wh2[:, m * O : (m + 1) * O],
                        rT[:, m * 1024 + hf * 512 : m * 1024 + (hf + 1) * 512],
                        start=(m == 0), stop=(m == KC - 1),
                    )
                nc.scalar.activation(
                    outs[:, hf * 512 : (hf + 1) * 512], ps2[:], Identity,
                    bias=bh2[:],
                )
            nc.sync.dma_start(out=d_out[:], in_=outs[:])

    nc.compile()
    return nc


def _split(w):
    hi = np.asarray(w, BF)
    lo = (np.asarray(w, np.float32) - hi.astype(np.float32)).astype(BF)
    return hi, lo


def _prep_inputs(times, coeffs, final_index, dW, W_init, b_init, W1, b1, W2,
                 b2, Wg, bg, Wh1, bh1, Wh2, bh2):
    f32 = np.float32
    times = np.asarray(times, f32)
    dt = f32(max(np.min(times[1:] - times[:-1]), f32(0.001)))
    sq = f32(np.sqrt(dt))

    def lhsT_layout(w):  # [H, H] -> [128, KC*H] with (k,m) tile at k*H+m*128
        return np.ascontiguousarray(
            np.asarray(w, f32).reshape(KC, 128, H).transpose(1, 0, 2).reshape(128, KC * H)
        )

    W1 = np.asarray(W1, f32)
    shared = {}
    for name, w in [("w1y", dt * W1[:H]), ("w2", np.asarray(W2, f32)),
                    ("wg", dt * np.asarray(Wg, f32))]:
        hi, lo = _split(lhsT_layout(w))
        shared[f"{name}_hi"] = hi
        shared[f"{name}_lo"] = lo
    w1b = np.vstack([W1[H:], np.asarray(b1, f32)[None, :]])
    shared["w1b_hi"], shared["w1b_lo"] = _split(w1b)
    shared["wini"] = np.ascontiguousarray(
        np.vstack([np.asarray(W_init, f32), np.asarray(b_init, f32)[None, :]]) / dt
    )
    b2hi, b2lo = _split(np.asarray(b2, f32)[None, :])
    bghi, bglo = _split(np.asarray(bg, f32)[None, :])
    shared["bias2"] = np.ascontiguousarray(np.vstack([b2hi, b2lo]))
    shared["biasg"] = np.ascontiguousarray(np.vstack([bghi, bglo]))
    ones = np.zeros((2, BL), f32)
    ones[0] = 1.0
    ones[1] = 1.0  # row 1 multiplies the lo bias row
    shared["ones2"] = ones.astype(BF)
    shared["wh1"] = lhsT_layout(dt * np.asarray(Wh1, f32))
    shared["wh2"] = np.ascontiguousarray(
        np.asarray(Wh2, f32).reshape(KC, 128, O).transpose(1, 0, 2).reshape(128, KC * O)
    )
    shared["bh1t"] = np.ascontiguousarray(np.asarray(bh1, f32).reshape(KC, 128).T)
    shared["bh2t"] = np.asarray(bh2, f32).reshape(O, 1)

    coeffs = np.asarray(coeffs, f32)  # [B, T, C]
    dW = np.asarray(dW, f32)  # [NT_full, B, H]
    dw_scale = f32(0.5 * sq / dt)
    in_maps = []
    NTP = T
    for c in range(NCORES):
        bs = slice(c * BL, (c + 1) * BL)
        xt = np.empty((T, C + 1, BL), f32)
        xt[:, :C, :] = coeffs[bs].transpose(1, 2, 0)
        xt[:, C, :] = 1.0
        # per-t packed [x~hi_t | x~lo_t] feature-major, zero pad past NT
        xhi, xlo = _split(xt)  # [T, C+1, BL] each
        xall2 = np.zeros((C + 1, NTP, 2, BL), BF)
        xall2[:, :NT, 0, :] = xhi[:NT].transpose(1, 0, 2)
        xall2[:, :NT, 1, :] = xlo[:NT].transpose(1, 0, 2)
        xall2 = np.ascontiguousarray(xall2.reshape(C + 1, NTP * 2 * BL))
        dwc = (dW[:NT, bs, :] * dw_scale).transpose(0, 2, 1)  # [NT, H, BL]
        dwc = np.ascontiguousarray(
            dwc.reshape(NT, KC, 128, BL).transpose(0, 2, 1, 3).reshape(NT, 128, KC * BL)
        )
        in_maps.append(
            {"xall2": xall2, "x0": np.ascontiguousarray(xt[0]), "dw": dwc,
             **shared}
        )
    return in_maps


def kernel(**inputs):
    global _BUILT
    if _BUILT is None:
        _BUILT = _build_nc()
    nc = _BUILT
    in_maps = _prep_inputs(**inputs)
    res = run_bass_kernel_spmd(nc, in_maps, core_ids=list(range(NCORES)))
    out = np.empty((B, OUT_TIME, O), np.float32)
    for c, r in enumerate(res.results):
        out[c * BL : (c + 1) * BL] = (
            r["out"].reshape(O, OUT_TIME, BL).transpose(2, 1, 0)
        )
    return out
